# revision 10
# baseline (speedup 1.0000x reference)
"""Distributed Trainium2 kernel for nn_Attention_77137612636887.

Full inputs -> full output. Sharding: 8 cores = 4 batches x 2 head-groups
(6 heads each). Each core runs QKV projection + attention for its heads and
a partial output projection over its 384 ctx dims; the host sums the two
partial projections per batch (row-sharded proj reduce) and concatenates
batches. Bias is added on the even core of each pair (via its bias input).

v2 schedule: query-chunk-OUTER, head-pair-inner attention blocks. Per-qc
normalization (reciprocal_approx_fast), broadcast, output projection and
output DMA are emitted immediately after each qc's last pair so they fill
PE slack under the ACT-paced attention of subsequent blocks. V-projection
tiles and next-pair QKV tiles are emitted interleaved inside attention
kb-groups so the scalar engine (softmax exp - the roofline at ~1.1ns/elem)
starts within a few us of kernel start and never starves. Scores matmuls
are explicitly row-tiled (K=64 pairs at tile_position (0,0)/(64,0), PSUM
banks split) so both heads' scores stream concurrently.
"""

import os
import sys

for _p in ("/opt/trn_rl_repo", "/root/.axon_site/_ro/trn_rl_repo"):
    if os.path.isdir(_p) and _p not in sys.path:
        sys.path.insert(0, _p)

import ml_dtypes
import numpy as np

import concourse.mybir as mybir
import concourse.tile as tile
from concourse import bacc
from concourse.bass_utils import run_bass_kernel_spmd

B, N, C, H, Dh = 4, 2048, 768, 12, 64
SCALE = Dh**-0.5
HPC = H // 2  # heads per core (6)
NPAIR = HPC // 2  # head pairs per core (3)
CSH = HPC * Dh  # ctx dims per core (384)
QC = 512  # query chunk (columns per score matmul)
NQC = N // QC  # 4
KB = 128  # key block
NKB = N // KB  # 16
KT = C // 128  # contraction subtiles for QKV (6)

F32 = mybir.dt.float32
BF16 = mybir.dt.bfloat16
BF16NP = ml_dtypes.bfloat16

EXP = mybir.ActivationFunctionType.Exp


def build_nc():
    nc = bacc.Bacc("TRN2", target_bir_lowering=False, debug=False, num_devices=8)

    xt_e = nc.declare_dram_parameter("xt", [128, KT, N], BF16, isOutput=False)
    wq_e = nc.declare_dram_parameter("wq", [128, KT, CSH], BF16, isOutput=False)
    wk_e = nc.declare_dram_parameter("wk", [128, KT, CSH], BF16, isOutput=False)
    wv_e = nc.declare_dram_parameter("wv", [128, KT, CSH], BF16, isOutput=False)
    wp_e = nc.declare_dram_parameter("wp", [128, NPAIR, C], BF16, isOutput=False)
    bias_e = nc.declare_dram_parameter("bias", [128, C], F32, isOutput=False)
    sel_e = nc.declare_dram_parameter("sel", [65, NPAIR, Dh], BF16, isOutput=False)
    out_e = nc.declare_dram_parameter("out", [N, C], F32, isOutput=True)

    with tile.TileContext(nc) as tc:
        with (
            tc.tile_pool(name="persist", bufs=1) as persist,
            tc.tile_pool(name="work", bufs=3) as work,
        ):
            # ---- persistent SBUF tensors ----
            xt_sb = persist.tile([128, KT, N], BF16, tag="xt")
            wq_sb = persist.tile([128, KT, CSH], BF16, tag="wq")
            wk_sb = persist.tile([128, KT, CSH], BF16, tag="wk")
            wv_sb = persist.tile([128, KT, CSH], BF16, tag="wv")
            wp_sb = persist.tile([128, NPAIR, C], BF16, tag="wp")
            bias_sb = persist.tile([128, C], F32, tag="bias")
            q_sb = persist.tile([128, NPAIR, N], BF16, tag="q")
            k_sb = persist.tile([128, NPAIR, N], BF16, tag="k")
            # v in natural [token, feat] layout: 64 v dims + ones col (65th)
            v_sb = persist.tile([128, NKB, HPC, 66], BF16, tag="v")
            cu_sb = persist.tile([128, NPAIR, N], BF16, tag="cu")
            ctx_sb = persist.tile([128, NPAIR, N], BF16, tag="ctx")
            # Z rows parked at partition 32*p (pair p), ping/pong over qc%2
            zall_sb = persist.tile([65, 2, 2, QC], F32, tag="zall")
            rzf_sb = persist.tile([65, 2, 2, QC], F32, tag="rzf")
            rz_sb = persist.tile([65, 2, 2, QC], BF16, tag="rz")
            sel_sb = persist.tile([65, NPAIR, Dh], BF16, tag="sel")
            warm_sb = persist.tile([128, 128], BF16, tag="warm")

            # ---- input DMAs, contiguous layouts, criticality order:
            # xt chunk 0 + wq + wk feed the first scores; rest stream behind
            nc.sync.dma_start(out=xt_sb[:, :, 0:QC], in_=xt_e[:, :, 0:QC])
            nc.sync.dma_start(out=wq_sb[:], in_=wq_e[:])
            nc.sync.dma_start(out=wk_sb[:], in_=wk_e[:])
            for qc in range(1, NQC):
                ts = slice(qc * QC, (qc + 1) * QC)
                nc.sync.dma_start(out=xt_sb[:, :, ts], in_=xt_e[:, :, ts])
            nc.sync.dma_start(out=wv_sb[:], in_=wv_e[:])
            nc.sync.dma_start(out=wp_sb[:], in_=wp_e[:])
            nc.sync.dma_start(out=bias_sb[:], in_=bias_e[:])
            nc.sync.dma_start(out=sel_sb[:], in_=sel_e[:])
            nc.vector.memset(warm_sb[:], 0.0)
            nc.vector.memset(v_sb[:, :, :, Dh : Dh + 1], 1.0)
            # unused zall partitions must stay finite for reciprocal
            nc.vector.memset(zall_sb[:], 1.0)

            with (
                tc.tile_pool(name="psS", bufs=1, space="PSUM") as psS,
                tc.tile_pool(name="psPV", bufs=1, space="PSUM") as psPV,
                tc.tile_pool(name="psAux", bufs=1, space="PSUM") as psAux,
            ):
                # ---------- aux emitters (feed PE during ACT-paced spans) ----
                def emit_q(p, qc):
                    ms = slice(p * 128, (p + 1) * 128)
                    ts = slice(qc * QC, (qc + 1) * QC)
                    ps_q = psAux.tile([128, QC], F32, tag="aux", bufs=2)
                    for kt in range(KT):
                        nc.tensor.matmul(
                            ps_q,
                            lhsT=wq_sb[:, kt, ms],
                            rhs=xt_sb[:, kt, ts],
                            start=(kt == 0),
                            stop=(kt == KT - 1),
                        )
                    nc.vector.tensor_copy(out=q_sb[:, p, ts], in_=ps_q[:])

                def emit_k(p, ck):
                    ms = slice(p * 128, (p + 1) * 128)
                    ts = slice(ck * QC, (ck + 1) * QC)
                    ps_k = psAux.tile([128, QC], F32, tag="aux", bufs=2)
                    for kt in range(KT):
                        nc.tensor.matmul(
                            ps_k,
                            lhsT=wk_sb[:, kt, ms],
                            rhs=xt_sb[:, kt, ts],
                            start=(kt == 0),
                            stop=(kt == KT - 1),
                        )
                    nc.vector.tensor_copy(out=k_sb[:, p, ts], in_=ps_k[:])

                def emit_v_tb(tb):
                    bs = slice(tb * KB, (tb + 1) * KB)
                    ps_v = psAux.tile([128, QC], F32, tag="aux", bufs=2, name=f"psv{tb}")[:, :CSH]
                    for kt in range(KT):
                        nc.tensor.matmul(
                            ps_v,
                            lhsT=xt_sb[:, kt, bs],
                            rhs=wv_sb[:, kt, :],
                            start=(kt == 0),
                            stop=(kt == KT - 1),
                        )
                    nc.vector.tensor_copy(
                        out=v_sb[:, tb, :, 0:Dh],
                        in_=ps_v[:].rearrange("p (h d) -> p h d", h=HPC),
                    )

                def emit_norm_proj(qc):
                    """reciprocal + broadcast + ctx mul + proj for chunk qc."""
                    ts = slice(qc * QC, (qc + 1) * QC)
                    qm = qc % 2
                    # 1/Z for all 3 pairs x 2 heads of this qc in two DVE ops
                    nc.vector.reciprocal_approx_fast(
                        out=rzf_sb[:, qm, :, :], in_=zall_sb[:, qm, :, :]
                    )
                    with nc.allow_low_precision(reason="softmax 1/Z in bf16"):
                        nc.vector.tensor_copy(
                            out=rz_sb[:, qm, :, :], in_=rzf_sb[:, qm, :, :]
                        )
                    for p in range(NPAIR):
                        bc = psAux.tile([128, QC], F32, tag="aux", bufs=2, name=f"bc{qc}_{p}")
                        nc.tensor.matmul(
                            bc[0:64, :],
                            lhsT=sel_sb[:, p, :],
                            rhs=rz_sb[:, qm, 0, :],
                            start=True,
                            stop=True,
                        )
                        nc.tensor.matmul(
                            bc[64:128, :],
                            lhsT=sel_sb[:, p, :],
                            rhs=rz_sb[:, qm, 1, :],
                            start=True,
                            stop=True,
                        )
                        nc.vector.tensor_mul(
                            out=ctx_sb[:, p, ts], in0=cu_sb[:, p, ts], in1=bc[:]
                        )
                    # partial output projection for this qc's 4 token blocks
                    for tb in range(qc * NQC, (qc + 1) * NQC):
                        bs = slice(tb * KB, (tb + 1) * KB)
                        for fs in (slice(0, 512), slice(512, 768)):
                            fw = fs.stop - fs.start
                            ps_o = psAux.tile([128, QC], F32, tag="aux", bufs=2, name=f"pso{tb}_{fs.start}")[:, :fw]
                            for p3 in range(NPAIR):
                                nc.tensor.matmul(
                                    ps_o,
                                    lhsT=ctx_sb[:, p3, bs],
                                    rhs=wp_sb[:, p3, fs],
                                    start=(p3 == 0),
                                    stop=(p3 == NPAIR - 1),
                                )
                            ob = work.tile(
                                [128, QC], F32, tag="ob", bufs=4, name=f"ob{tb}{fw}"
                            )[:, :fw]
                            nc.vector.tensor_add(
                                out=ob[:], in0=ps_o[:], in1=bias_sb[:, fs]
                            )
                            nc.sync.dma_start(out=out_e[bs, fs], in_=ob[:])

                # ---------- attention block ----------
                def emit_pv(item, pv_A, pv_B, hA, hB):
                    kb, p_ab = item
                    nc.tensor.matmul(
                        pv_A[0:65, :],
                        lhsT=v_sb[:, kb, hA, 0:65],
                        rhs=p_ab[:, 0:QC],
                        start=(kb == 0),
                        stop=(kb == NKB - 1),
                    )
                    nc.tensor.matmul(
                        pv_B[0:65, :],
                        lhsT=v_sb[:, kb, hB, 0:65],
                        rhs=p_ab[:, QC : 2 * QC],
                        start=(kb == 0),
                        stop=(kb == NKB - 1),
                    )

                def attention_block(qc, p, feed):
                    """feed: list of zero-arg emitters to interleave, ~1/kb."""
                    hA, hB = 2 * p, 2 * p + 1
                    qm = qc % 2
                    ts = slice(qc * QC, (qc + 1) * QC)
                    pv_AB = psPV.tile([128, 2, QC], F32, tag="pvAB", bufs=1)
                    pv_A = pv_AB[:, 0, :]
                    pv_B = pv_AB[:, 1, :]
                    pipe = []
                    for kb in range(NKB):
                        if feed:
                            feed.pop(0)()
                        if len(feed) > 8:
                            feed.pop(0)()
                        ks = slice(kb * KB, (kb + 1) * KB)
                        s_ab = psS.tile([128, 2 * QC], F32, tag="s", bufs=2)
                        nc.tensor.matmul(
                            s_ab[:, 0:QC],
                            lhsT=k_sb[0:64, p, ks],
                            rhs=q_sb[0:64, p, ts],
                            start=True,
                            stop=True,
                            tile_position=(0, 0),
                        )
                        nc.tensor.matmul(
                            s_ab[:, QC : 2 * QC],
                            lhsT=k_sb[64:128, p, ks],
                            rhs=q_sb[64:128, p, ts],
                            start=True,
                            stop=True,
                            tile_position=(64, 0),
                        )
                        p_ab = work.tile([128, 2 * QC], BF16, tag="p_ab", bufs=6)
                        nc.scalar.activation(p_ab[:], s_ab[:], EXP, scale=SCALE)
                        pipe.append((kb, p_ab))
                        if len(pipe) == 3:
                            emit_pv(pipe.pop(0), pv_A, pv_B, hA, hB)
                    while pipe:
                        if feed:
                            feed.pop(0)()
                        emit_pv(pipe.pop(0), pv_A, pv_B, hA, hB)
                    # stash unnormalized ctx + Z rows
                    nc.vector.tensor_copy(out=cu_sb[0:64, p, ts], in_=pv_A[0:Dh, :])
                    nc.vector.tensor_copy(
                        out=cu_sb[64:128, p, ts], in_=pv_B[0:Dh, :]
                    )
                    nc.vector.tensor_copy(
                        out=zall_sb[32 * p : 32 * p + 1, qm, :, :],
                        in_=pv_AB[Dh : Dh + 1, :, :],
                    )

                # ---------- main schedule ----------
                # PE warmup: dummy matmuls chew the HAM cold window while the
                # first xt/wq/wk DMAs land (PE would otherwise sit idle, start
                # cold, and run the first QKV tiles at half clock)
                for w in range(20):
                    ps_w = psS.tile([128, 2 * QC], F32, tag="s", bufs=2, name=f"warm{w}")
                    nc.tensor.matmul(
                        ps_w[:, 0:128],
                        lhsT=warm_sb[:],
                        rhs=warm_sb[:],
                        start=True,
                        stop=True,
                    )
                # minimal head: q(p0,qc0) + k(p0,chunk0) before first scores
                emit_q(0, 0)
                emit_k(0, 0)

                # pending aux work, one item consumed per kb slot
                feed = []
                # remaining k chunks for p0, V tiles, then q(p0, qc1..3)
                for ck in range(1, NQC):
                    feed.append(lambda p=0, ck=ck: emit_k(p, ck))
                for tb in range(NKB):
                    feed.append(lambda tb=tb: emit_v_tb(tb))

                def queue_qk(p):
                    feed.append(lambda p=p: emit_q(p, 0))
                    for ck in range(NQC):
                        feed.append(lambda p=p, ck=ck: emit_k(p, ck))

                def queue_q(p, qc):
                    feed.append(lambda p=p, qc=qc: emit_q(p, qc))

                queue_qk(1)

                for qc in range(NQC):
                    for p in range(NPAIR):
                        if qc == 0 and p == 1:
                            queue_qk(2)
                        if p == 2:
                            for pn in range(NPAIR):
                                if qc + 1 < NQC:
                                    queue_q(pn, qc + 1)
                        attention_block(qc, p, feed)
                    emit_norm_proj(qc)

    nc.finalize()
    return nc


def make_in_maps(x, w_qkv, b_proj, w_proj):
    """Per-core inputs. Core c: batch c//2, head-group c%2."""
    wq_full = w_qkv[0 * C : 1 * C]  # [H*Dh, C]
    wk_full = w_qkv[1 * C : 2 * C]
    wv_full = w_qkv[2 * C : 3 * C]

    sel = np.zeros((65, NPAIR, Dh), BF16NP)
    for p in range(NPAIR):
        sel[32 * p, p, :] = 1.0

    in_maps = []
    for c in range(8):
        b, hg = c // 2, c % 2
        heads = [hg * HPC + i for i in range(HPC)]
        rows = np.concatenate([np.arange(h * Dh, (h + 1) * Dh) for h in heads])
        # device-native layouts: [C, *] split as C=(kt p) -> [p, kt, *]
        xt = np.ascontiguousarray(
            x[b].T.reshape(KT, 128, N).transpose(1, 0, 2)
        ).astype(BF16NP)  # [128, KT, N]
        wq = np.ascontiguousarray(
            wq_full[rows].T.reshape(KT, 128, CSH).transpose(1, 0, 2)
        ).astype(BF16NP)
        wk = np.ascontiguousarray(
            wk_full[rows].T.reshape(KT, 128, CSH).transpose(1, 0, 2)
        ).astype(BF16NP)
        wv = np.ascontiguousarray(
            wv_full[rows].T.reshape(KT, 128, CSH).transpose(1, 0, 2)
        ).astype(BF16NP)
        wp = np.ascontiguousarray(
            w_proj[:, rows].T.reshape(NPAIR, 128, C).transpose(1, 0, 2)
        ).astype(BF16NP)  # [128, NPAIR, C]
        if hg == 0:
            bias = np.tile(b_proj[None, :], (128, 1)).astype(np.float32)
        else:
            bias = np.zeros((128, C), np.float32)
        in_maps.append(
            {"xt": xt, "wq": wq, "wk": wk, "wv": wv, "wp": wp, "bias": bias, "sel": sel}
        )
    return in_maps


_NC = None


def kernel(x, xpos=None, w_qkv=None, w_proj=None, b_proj=None, **kw):
    global _NC
    x = np.asarray(x, np.float32)
    w_qkv = np.asarray(w_qkv, np.float32)
    w_proj = np.asarray(w_proj, np.float32)
    b_proj = np.asarray(b_proj, np.float32)

    if _NC is None:
        _NC = build_nc()
    in_maps = make_in_maps(x, w_qkv, b_proj, w_proj)
    res = run_bass_kernel_spmd(_NC, in_maps, core_ids=list(range(8)))
    out = np.empty((B, N, C), np.float32)
    for b in range(B):
        out[b] = res.results[2 * b]["out"] + res.results[2 * b + 1]["out"]
    return out
